# revision 2
# baseline (speedup 1.0000x reference)
"""Trainium2 Bass kernel for StyleGAN2-style modulated conv2d (ModConv2D).

Reference computation (per sample b):
    w      = kernel * (style[b] + 1)                 # modulate [3,3,Cin,Cout]
    w      = w / sqrt(sum(w^2, (kh,kw,Cin)) + 1e-8)  # demodulate per Cout
    y[b]   = conv2d_same(x[b], w)

Sharding: data-parallel over batch — 16 samples across 8 NeuronCores,
2 samples per core; the base kernel is replicated.

Device algorithm per core (2 samples):
  - conv as 9-tap accumulated matmuls: psum[cout,pix] += w[t,cin,cout]^T @
    xT[cin, pix+off].  x is held channel-major FLAT ([cin, cc, 64+4096+80]
    fp16) with zero guard rows; horizontal (dx=+-1) taps use column-split
    matmuls (N=504, strided psum out) so row wrap never leaks.
  - ALL transposes (x ingest and y output) run on the DMA xbar so the PE
    does only conv matmuls.  x loads are cc-split ([128 pix, 4, 128 cin]
    fp16 tiles, contiguous) so the xbar input AP is contiguous.
  - demod factor d[cout] = rsqrt(sum_cin s^2 * K2 + 1e-8) in fp32 on device
    (K2 = sum_t kernel^2 once per core), applied as a per-partition scale on
    psum eviction (ACT).  Output staged fp16, cast back to fp32 by the
    store DMA (SWDGE).
"""

import numpy as np

B, H, W, CIN, COUT, KH, KW = 16, 64, 64, 256, 256, 3, 3
NCORES = 8
BPC = B // NCORES  # samples per core
T = KH * KW  # 9 taps
HWPIX = H * W  # 4096
PAD0 = 64  # zero pixels before the image
XLEN = PAD0 + HWPIX + 80  # 4240: multiple of 16 so xbar dest strides stay 32B-aligned

# tap order: dx=0 taps first so the first matmul of each psum group writes all
# 512 columns with start=True
TAP_ORDER = [1, 4, 7, 0, 3, 6, 2, 5, 8]

_CACHE = {}
LAST_EXEC_NS = None
LAST_MEAN_EXEC_NS = None


def _build_nc():
    from contextlib import ExitStack

    import concourse.bacc as bacc
    import concourse.bass as bass
    import concourse.mybir as mybir
    import concourse.tile as tile

    f32 = mybir.dt.float32
    bf16 = mybir.dt.float16  # fp16: same 1 cyc/row PE rate as bf16, 4x finer mantissa
    AF = mybir.ActivationFunctionType

    nc = bacc.Bacc("TRN2", target_bir_lowering=False, debug=False)

    x_d = nc.dram_tensor("x", [BPC, H, W, CIN], f32, kind="ExternalInput")
    s_d = nc.dram_tensor("style", [BPC, CIN], f32, kind="ExternalInput")
    k_d = nc.dram_tensor("kernel", [KH, KW, CIN, COUT], f32, kind="ExternalInput")
    y_d = nc.dram_tensor("y", [BPC, H, W, COUT], f32, kind="ExternalOutput")

    XB = H * W * CIN  # x/y sample stride (elements)
    KKW = CIN * COUT  # kernel tap stride

    def x_blk_ap(b, t8, cc):
        # [128 pix, 4 sblk, 128 cin] starting at pixel (t8*4)*128, cin chunk cc
        off = b * XB + t8 * 4 * 128 * CIN + cc * 128
        return bass.AP(x_d, off, [[CIN, 128], [128 * CIN, 4], [1, 128]])

    def y_blk_ap(b, t8):
        off = b * XB + t8 * 4 * 128 * COUT
        return bass.AP(y_d, off, [[COUT, 128], [128 * COUT, 4], [1, COUT]])

    def k_tap_ap(cc, t):
        # [128 cin, 256 cout] for one tap
        return bass.AP(k_d, t * KKW + cc * 128 * COUT, [[COUT, 128], [1, COUT]])

    with tile.TileContext(nc) as tc, ExitStack() as ctx:
        singles = ctx.enter_context(tc.tile_pool(name="singles", bufs=1))
        tmp_pool = ctx.enter_context(tc.tile_pool(name="tmp", bufs=1))
        wpool = ctx.enter_context(tc.tile_pool(name="wpool", bufs=2))
        dpool = ctx.enter_context(tc.tile_pool(name="dpool", bufs=2))
        srow_pool = ctx.enter_context(tc.tile_pool(name="srow", bufs=2))
        xpool = ctx.enter_context(tc.tile_pool(name="xpool", bufs=2))
        xtpool = ctx.enter_context(tc.tile_pool(name="xt", bufs=2 * 16))
        ospool = ctx.enter_context(tc.tile_pool(name="osb", bufs=6))
        obpool = ctx.enter_context(tc.tile_pool(name="ob", bufs=4))
        pconv = ctx.enter_context(tc.tile_pool(name="pconv", bufs=6, space="PSUM"))
        psmall = ctx.enter_context(tc.tile_pool(name="psmall", bufs=1, space="PSUM"))

        # style rows + per-tap kernel loads (conv tap order; the modulated
        # weights gate the conv ramp), on the scalar HWDGE ring (sync ring
        # carries the ingest xbar transposes)
        srows = []
        for b in range(BPC):
            srow = srow_pool.tile([1, CIN], f32, tag="srow")
            nc.scalar.dma_start(out=srow, in_=s_d.ap()[b : b + 1, :])
            srows.append(srow)
        kbase = singles.tile([128, 2, T, COUT], f32)
        for ti, t in enumerate(TAP_ORDER):
            for cc in range(2):
                nc.scalar.dma_start(out=kbase[:, cc, t], in_=k_tap_ap(cc, t))

        # all x loads (cast fp32->fp16, SWDGE) issued upfront, cc-split so the
        # xbar transpose input is a contiguous [128, 512] tile
        xts = [[[None] * 2 for _ in range(8)] for _ in range(BPC)]

        def load_xtmp(b, t8, cc):
            xtmp = xtpool.tile(
                [128, 4, 128], bf16, tag="xtmp", name=f"xtmp_{b}_{t8}_{cc}"
            )
            nc.gpsimd.dma_start(out=xtmp, in_=x_blk_ap(b, t8, cc))
            xts[b][t8][cc] = xtmp

        for b in range(BPC):
            for t8 in range(8):
                for cc in range(2):
                    load_xtmp(b, t8, cc)

        ones1 = singles.tile([1, 1], f32)
        nc.vector.memset(ones1, 1.0)
        eps_sb = singles.tile([128, 1], f32)
        nc.vector.memset(eps_sb, 1e-8)

        # K2[cin, cout] = sum_t kernel^2  (once per core)
        k2 = singles.tile([128, 2, COUT], f32)
        for cc in range(2):
            k2tmp = tmp_pool.tile([128, T, COUT], f32)
            nc.vector.tensor_mul(k2tmp, kbase[:, cc], kbase[:, cc])
            nc.vector.reduce_sum(
                out=k2[:, cc],
                in_=k2tmp.rearrange("p t c -> p c t"),
                axis=mybir.AxisListType.X,
            )

        # ---- modulation + demod factors for BOTH samples, upfront ----
        wbs, dsbs = [], []
        for b in range(BPC):
            srow1 = srow_pool.tile([1, CIN], f32, tag="srow1")
            nc.vector.tensor_scalar_add(srow1, srows[b], 1.0)

            smod = dpool.tile([128, 2], f32)  # (style+1) col-major per cc
            s2c = dpool.tile([128, 2], f32)
            for cc in range(2):
                pcol = psmall.tile([128, 1], f32, tag="psmall")
                nc.tensor.matmul(
                    pcol, srow1[:, cc * 128 : (cc + 1) * 128], ones1, start=True, stop=True
                )
                nc.vector.tensor_copy(out=smod[:, cc : cc + 1], in_=pcol)
            nc.vector.tensor_mul(s2c, smod, smod)

            # wb[cin, cc, t, cout] = kernel * (s+1), cast fp16, on ACT, per
            # tap in conv order so the first conv matmuls unblock early
            wb = wpool.tile([128, 2, T, COUT], bf16)
            for t in TAP_ORDER:
                for cc in range(2):
                    nc.scalar.activation(
                        wb[:, cc, t], kbase[:, cc, t], AF.Copy,
                        scale=smod[:, cc : cc + 1],
                    )
            wbs.append(wb)

            # sumsq[cout] = sum_cc s2c^T @ k2 -> [1, 256] -> demod d [128, 2]
            prow = psmall.tile([1, COUT], f32, tag="psmall")
            for cc in range(2):
                nc.tensor.matmul(
                    prow, s2c[:, cc : cc + 1], k2[:, cc], start=(cc == 0), stop=(cc == 1)
                )
            ssq_row = srow_pool.tile([1, COUT], f32, tag="ssq")
            nc.vector.tensor_copy(out=ssq_row, in_=prow)
            sqc = dpool.tile([128, 2], f32)
            for oc in range(2):
                pcol2 = psmall.tile([128, 1], f32, tag="psmall")
                nc.tensor.matmul(
                    pcol2, ssq_row[:, oc * 128 : (oc + 1) * 128], ones1, start=True, stop=True
                )
                nc.scalar.activation(sqc[:, oc : oc + 1], pcol2, AF.Sqrt, bias=eps_sb)
            d_sb = dpool.tile([128, 2], f32)
            nc.vector.reciprocal(d_sb, sqc)
            dsbs.append(d_sb)

        for b in range(BPC):
            wb = wbs[b]
            d_sb = dsbs[b]
            # x, channel-major flat: [128 cin, cc, PAD0 + 4096 + 80] bf16
            xflat = xpool.tile([128, 2, XLEN], bf16)
            nc.vector.memset(xflat[:, :, 0:PAD0], 0.0)
            nc.vector.memset(xflat[:, :, PAD0 + HWPIX : XLEN], 0.0)

            def transpose_block(t8):
                # xbar transpose: [128 pix, 512] fp16 -> [128 cin, 4, 128 pix]
                for cc in range(2):
                    dst = xflat[:, cc, PAD0 + 512 * t8 : PAD0 + 512 * (t8 + 1)]
                    nc.sync.dma_start_transpose(
                        out=dst.rearrange("p (s q) -> p s q", q=128),
                        in_=xts[b][t8][cc],
                    )

            def conv_tile(t8):
                # output pixels p0 .. p0+511, both cout chunks
                ob = obpool.tile([128, 4, COUT], bf16, tag="ob")
                p0 = t8 * 512
                for oc in range(2):
                    ps = pconv.tile([128, 512], f32, tag="pconv")
                    ps_r = ps.rearrange("p (r w) -> p r w", w=64)
                    i = 0
                    for t in TAP_ORDER:
                        dy, dx = t // 3 - 1, t % 3 - 1
                        base = PAD0 + p0 + 64 * dy
                        for cc in range(2):
                            lhsT = wb[:, cc, t, oc * 128 : (oc + 1) * 128]
                            xf = xflat[:, cc]
                            if dx == 0:
                                rhs = xf[:, base : base + 512]
                                out_ap = ps
                            elif dx == -1:
                                rhs = xf[:, base : base + 512].rearrange(
                                    "p (r w) -> p r w", w=64
                                )[:, :, 0:63]
                                out_ap = ps_r[:, :, 1:64]
                            else:  # dx == +1
                                rhs = xf[:, base + 1 : base + 513].rearrange(
                                    "p (r w) -> p r w", w=64
                                )[:, :, 0:63]
                                out_ap = ps_r[:, :, 0:63]
                            nc.tensor.matmul(
                                out_ap, lhsT, rhs, start=(i == 0), stop=(i == 17)
                            )
                            i += 1
                    o_sb = ospool.tile([128, 512], bf16, tag="osb")
                    nc.scalar.activation(o_sb, ps, AF.Copy, scale=d_sb[:, oc : oc + 1])
                    # output transpose on the DMA xbar, split across rings
                    eng = nc.sync if oc == 0 else nc.scalar
                    eng.dma_start_transpose(
                        out=ob[:, :, oc * 128 : (oc + 1) * 128], in_=o_sb
                    )
                nc.gpsimd.dma_start(out=y_blk_ap(b, t8), in_=ob)

            PF = 2  # transpose prefetch distance ahead of conv
            for t8 in range(PF):
                transpose_block(t8)
            for t8 in range(PF, 8):
                transpose_block(t8)
                conv_tile(t8 - PF)
            for t8 in range(8 - PF, 8):
                conv_tile(t8)

    nc.compile()
    return nc


def _get_nc():
    if "nc" not in _CACHE:
        _CACHE["nc"] = _build_nc()
    return _CACHE["nc"]


def kernel(x, style, kernel, _trace=False):
    global LAST_EXEC_NS, LAST_MEAN_EXEC_NS
    from concourse.bass_utils import run_bass_kernel_spmd

    x = np.ascontiguousarray(x, dtype=np.float32)
    style = np.ascontiguousarray(style, dtype=np.float32)
    kern = np.ascontiguousarray(kernel, dtype=np.float32)

    nc = _get_nc()
    in_maps = [
        {
            "x": x[i * BPC : (i + 1) * BPC],
            "style": style[i * BPC : (i + 1) * BPC],
            "kernel": kern,
        }
        for i in range(NCORES)
    ]
    res = run_bass_kernel_spmd(nc, in_maps, core_ids=list(range(NCORES)), trace=_trace)
    LAST_EXEC_NS = res.exec_time_ns
    LAST_MEAN_EXEC_NS = res.mean_exec_time_ns
    return np.concatenate([res.results[i]["y"] for i in range(NCORES)], axis=0)


# revision 4
# speedup vs baseline: 1.7607x; 1.7607x over previous
"""Trainium2 Bass kernel for StyleGAN2-style modulated conv2d (ModConv2D).

v5: 1D Winograd F(2,3) along width; input transform fused into the PE
ingest "transpose" via a 0/+-1 B-matrix (out = x_tile^T @ B puts all four
V points directly in PSUM), planar even/odd outputs re-interleaved for
free by strided store APs.  Cayman errata makes DVE/ACT SBUF ops ~2.3x
slower than spec, so per-element work on those engines is minimized.

Reference computation (per sample b):
    w      = kernel * (style[b] + 1)                 # modulate [3,3,Cin,Cout]
    w      = w / sqrt(sum(w^2, (kh,kw,Cin)) + 1e-8)  # demodulate per Cout
    y[b]   = conv2d_same(x[b], w)

Winograd (per output pair (2t, 2t+1) in a row, d_k = x[row, 2t-1+k]):
    V0 = d0-d2, V1 = d1+d2, V2 = d2-d1, V3 = d1-d3          (input transform)
    W0 = g0, W1 = (g0+g1+g2)/2, W2 = (g0-g1+g2)/2, W3 = g2  (per kh row g)
    m_p[cout] = sum_dy sum_cin W_p[dy,cin,cout] * V_p[cin, row=oy+dy-1]
    y[2t] = m0+m1+m2,  y[2t+1] = m1-m2-m3                   (output transform)

Key engine placement (lessons from v1/v3 traces):
  - PE ingest transposes use a PERMUTATION matrix (not identity) that
    de-interleaves even/odd pixels per row, so every V-transform op on DVE
    is unit-stride (v3's stride-2 ops ran at ~2x cost).
  - V is computed by DVE DIRECTLY from the transpose psum (xflat staging
    buffer eliminated entirely).
  - Winograd matmuls at N=512 (16-row blocks), 4 points split into two
    2-bank psum tiles (points 0,1 / 2,3), so LDWEIGHTS stays hidden.
  - Demod sum uses fp16 squares + accumulating PE matmuls over taps
    (v3's TENSOR_REDUCE cost 8us of DVE and gated the pipeline).
  - Inverse-transform combines run on GPSIMD (DVE was the v3 bottleneck).
  - ACT: psum evictions with demod scale + modulation.  Output transposes
    on the DMA xbar; x loads / y stores on SWDGE with dtype cast.
"""

import numpy as np

B, H, W, CIN, COUT, KH, KW = 16, 64, 64, 256, 256, 3, 3
NCORES = 8
BPC = B // NCORES  # samples per core
T = KH * KW  # 9 taps
HWPIX = H * W  # 4096
NT = W // 2  # 32 winograd tiles per row
VROWS = H + 2  # 66 plane rows: [0]=row -1 pad, [65]=row 64 pad
RB = 16  # output rows per conv block
NBLK = H // RB  # 4

_CACHE = {}
LAST_EXEC_NS = None
LAST_MEAN_EXEC_NS = None


def _build_nc():
    from contextlib import ExitStack

    import concourse.bacc as bacc
    import concourse.bass as bass
    import concourse.mybir as mybir
    import concourse.tile as tile
    from concourse.masks import make_identity

    f32 = mybir.dt.float32
    bf16 = mybir.dt.float16
    AF = mybir.ActivationFunctionType

    nc = bacc.Bacc("TRN2", target_bir_lowering=False, debug=False)

    x_d = nc.dram_tensor("x", [BPC, H, W, CIN], f32, kind="ExternalInput")
    s_d = nc.dram_tensor("style", [BPC, CIN], f32, kind="ExternalInput")
    k_d = nc.dram_tensor("kernel", [KH, KW, CIN, COUT], f32, kind="ExternalInput")
    y_d = nc.dram_tensor("y", [BPC, H, W, COUT], f32, kind="ExternalOutput")

    XB = H * W * CIN  # x/y sample stride (elements)
    KKW = CIN * COUT  # kernel tap stride

    def x_blk_ap(b, t8):
        # [128 pix, 4 sblk, 256 cin] starting at pixel t8*512
        off = b * XB + t8 * 4 * 128 * CIN
        return bass.AP(x_d, off, [[CIN, 128], [128 * CIN, 4], [1, CIN]])

    def y_par_ap(b, blk, par):
        # [128 q, 4 sblk, 256 cout]: even/odd pixels (2q+par) of block blk
        off = b * XB + blk * 1024 * COUT + par * COUT
        return bass.AP(y_d, off, [[2 * COUT, 128], [256 * COUT, 4], [1, COUT]])

    def k_tap_ap(cc, t):
        # [128 cin, 256 cout] for one tap
        return bass.AP(k_d, t * KKW + cc * 128 * COUT, [[COUT, 128], [1, COUT]])

    with tile.TileContext(nc) as tc, ExitStack() as ctx:
        singles = ctx.enter_context(tc.tile_pool(name="singles", bufs=1))
        wpool = ctx.enter_context(tc.tile_pool(name="wpool", bufs=1))
        dpool = ctx.enter_context(tc.tile_pool(name="dpool", bufs=2))
        srow_pool = ctx.enter_context(tc.tile_pool(name="srow", bufs=2))
        kwpool = ctx.enter_context(tc.tile_pool(name="kw", bufs=1))
        xtpool = ctx.enter_context(tc.tile_pool(name="xt", bufs=24))
        vpool = ctx.enter_context(tc.tile_pool(name="vpool", bufs=2))
        evpool = ctx.enter_context(tc.tile_pool(name="ev", bufs=4))
        gtmp = ctx.enter_context(tc.tile_pool(name="gtmp", bufs=2))
        ospool = ctx.enter_context(tc.tile_pool(name="osb", bufs=3))
        obpool = ctx.enter_context(tc.tile_pool(name="ob", bufs=3))
        paux = ctx.enter_context(tc.tile_pool(name="paux", bufs=2, space="PSUM"))
        pwino = ctx.enter_context(tc.tile_pool(name="pwino", bufs=2, space="PSUM"))

        # ---- loads: style + kernel taps split across both HWDGE rings ----
        srows = []
        for b in range(BPC):
            srow = srow_pool.tile([1, CIN], f32, tag="srow")
            nc.scalar.dma_start(out=srow, in_=s_d.ap()[b : b + 1, :])
            srows.append(srow)
        # kernel loaded once, cast to fp16 by the SWDGE (one DMA per cc)
        k16 = singles.tile([128, 2, T, COUT], bf16)

        def load_kcc(cc):
            ap = bass.AP(
                k_d, cc * 128 * COUT, [[COUT, 128], [KKW, T], [1, COUT]]
            )
            nc.gpsimd.dma_start(out=k16[:, cc], in_=ap)

        # x loads (cast fp32->fp16, SWDGE) issued upfront
        xts = [[None] * 8 for _ in range(BPC)]

        def load_xtmp(b, t8):
            xtmp = xtpool.tile([128, 4, CIN], bf16, tag="xtmp", name=f"xtmp_{b}_{t8}")
            nc.gpsimd.dma_start(out=xtmp, in_=x_blk_ap(b, t8))
            xts[b][t8] = xtmp

        load_xtmp(0, 0)
        load_xtmp(0, 1)
        load_kcc(0), load_kcc(1)
        # winograd input-transform matrix bw[pix, (r, point, t)]: the ingest
        # "transpose" computes x_tile^T @ bw, landing all four V points for
        # the tile's two rows directly in psum (horizontal pads included)
        ident_b = singles.tile([128, 128], bf16)
        make_identity(nc, ident_b)
        bw = singles.tile([128, 2, 4, NT], bf16)
        for r in range(2):
            Dv = ident_b[:, 64 * r : 64 * r + 64].rearrange("p (q t) -> p q t", t=2)
            E_, O_ = Dv[:, :, 0], Dv[:, :, 1]
            nc.vector.tensor_sub(bw[:, r, 0, 1:32], O_[:, 0:31], O_[:, 1:32])
            nc.vector.tensor_scalar_mul(bw[:, r, 0, 0:1], O_[:, 0:1], -1.0)
            nc.vector.tensor_add(bw[:, r, 1], E_, O_)
            nc.vector.tensor_sub(bw[:, r, 2], O_, E_)
            nc.vector.tensor_sub(bw[:, r, 3, 0:31], E_[:, 0:31], E_[:, 1:32])
            nc.vector.tensor_copy(out=bw[:, r, 3, 31:32], in_=E_[:, 31:32])
        # fold the winograd 1/2 into the V1/V2 columns (exact in fp16)
        nc.vector.tensor_scalar_mul(bw[:, :, 1:3, :], bw[:, :, 1:3, :], 0.5)
        bw_flat = bw.rearrange("p r q t -> p (r q t)")
        load_xtmp(0, 2)
        for b in range(BPC):
            for t8 in range(8):
                if xts[b][t8] is None:
                    load_xtmp(b, t8)

        ones1 = singles.tile([1, 1], f32)
        nc.vector.memset(ones1, 1.0)
        eps_sb = singles.tile([128, 1], f32)
        nc.vector.memset(eps_sb, 1e-8)

        # squared taps for the demod sum (fp16; accumulated on PE)
        ksq = kwpool.tile([128, 2, T, COUT], bf16)

        smods, s2c16s = [], []

        def style_prep(b):
            # (style+1) as an SBUF column pair via tiny PE transposes
            srow1 = srow_pool.tile([1, CIN], f32, tag="srow1")
            nc.vector.tensor_scalar_add(srow1, srows[b], 1.0)
            smod = dpool.tile([128, 2], f32, tag="smod")
            for cc in range(2):
                pcol = paux.tile([128, 1], f32, tag="pxt")
                nc.tensor.matmul(
                    pcol, srow1[:, cc * 128 : (cc + 1) * 128], ones1,
                    start=True, stop=True,
                )
                nc.vector.tensor_copy(out=smod[:, cc : cc + 1], in_=pcol)
            s2c = dpool.tile([128, 2], bf16, tag="s2c")
            nc.vector.tensor_mul(s2c, smod, smod)
            smods.append(smod), s2c16s.append(s2c)

        # winograd weights, style-free (modulation by (1+s)[cin] rides the V
        # psum eviction): points 0/3 index k16 directly; 1/2 are fp16 combos
        wb12 = wpool.tile([128, 2, 2, KH, COUT], bf16, tag="wb12")

        def build_weights():
            for cc in range(2):
                kb = k16[:, cc].rearrange("p (dy kw) c -> p kw dy c", kw=3)
                t02 = gtmp.tile([128, KH, COUT], bf16, tag="t02", bufs=1)
                nc.vector.tensor_add(t02, kb[:, 0], kb[:, 2])
                nc.vector.tensor_add(wb12[:, cc, 0], t02, kb[:, 1])
                nc.vector.tensor_sub(wb12[:, cc, 1], t02, kb[:, 1])

        def lhsT(cc, p, dy, oc):
            sl = slice(oc * 128, (oc + 1) * 128)
            if p == 0:
                return k16[:, cc, dy * 3 + 0, sl]
            if p == 3:
                return k16[:, cc, dy * 3 + 2, sl]
            return wb12[:, cc, p - 1, dy, sl]

        dsbs = [None] * BPC

        def demod(b):
            # sumsq[cout] = sum_t sum_cin s2*ksq  (accumulating PE matmuls)
            prow = paux.tile([1, COUT], f32, tag="pxt")
            i = 0
            for cc in range(2):
                for t in range(T):
                    nc.tensor.matmul(
                        prow, s2c16s[b][:, cc : cc + 1], ksq[:, cc, t],
                        start=(i == 0), stop=(i == 2 * T - 1),
                    )
                    i += 1
            ssq_row = srow_pool.tile([1, COUT], f32, tag="ssq")
            nc.vector.tensor_copy(out=ssq_row, in_=prow)
            sqc = dpool.tile([128, 2], f32, tag="sqc")
            for oc in range(2):
                pcol2 = paux.tile([128, 1], f32, tag="pxt")
                nc.tensor.matmul(
                    pcol2, ssq_row[:, oc * 128 : (oc + 1) * 128], ones1,
                    start=True, stop=True,
                )
                nc.scalar.activation(sqc[:, oc : oc + 1], pcol2, AF.Sqrt, bias=eps_sb)
            d_sb = dpool.tile([128, 2], f32, tag="dsb")
            nc.vector.reciprocal(d_sb, sqc)
            dsbs[b] = d_sb

        vts = [None] * BPC

        def start_sample(b):
            vt = vpool.tile([128, 2, VROWS, 4, NT], bf16, tag="vt", name=f"vt_{b}")
            vts[b] = vt
            # vertical SAME-pad rows of the V planes
            nc.vector.memset(vt[:, :, 0:1, :, :], 0.0)
            nc.vector.memset(vt[:, :, VROWS - 1 : VROWS, :, :], 0.0)

        def ivchunk(b, t8):
            # ingest transposes (PE, de-interleaving perm) + V transform (DVE)
            # for image rows 8*t8 .. 8*t8+7  (plane rows 8*t8+1 .. 8*t8+8)
            xtmp = xts[b][t8]
            vt = vts[b]
            for cc in range(2):
                pxt_t = paux.tile([128, 4, 2, 4, NT], f32, tag="pxt")
                for s in range(4):
                    nc.tensor.matmul(
                        pxt_t[:, s],
                        xtmp[:, s, cc * 128 : (cc + 1) * 128],
                        bw_flat,
                        start=True,
                        stop=True,
                    )
                # psum iteration (s, r, p, t) == vt iteration (row, p, t);
                # eviction applies the per-cin modulation scale (1+s)
                vo = vt[:, cc, 8 * t8 + 1 : 8 * t8 + 9, :, :]
                sc = smods[b][:, cc : cc + 1]
                if cc == 0:
                    nc.scalar.activation(vo, pxt_t, AF.Copy, scale=sc)
                else:
                    nc.vector.tensor_scalar_mul(vo, pxt_t, sc)

        def conv_blk(b, blk, mid_hook=None):
            vt = vts[b]
            ob = obpool.tile([128, 4, COUT], bf16, tag="ob")
            obo = obpool.tile([128, 4, COUT], bf16, tag="obo")
            for oc in range(2):
                pwA = pwino.tile([128, 2, 512], f32, tag="pwino")  # points 0,1
                pwB = pwino.tile([128, 2, 512], f32, tag="pwino")  # points 2,3
                psl = {0: pwA[:, 0], 1: pwA[:, 1], 2: pwB[:, 0], 3: pwB[:, 1]}
                for pi, p in enumerate((0, 3, 1, 2)):
                    i = 0
                    for dy in range(KH):
                        for cc in range(2):
                            rhs = vt[:, cc, RB * blk + dy : RB * blk + dy + RB, p, :]
                            nc.tensor.matmul(
                                psl[p],
                                lhsT(cc, p, dy, oc),
                                rhs,
                                start=(i == 0),
                                stop=(i == 5),
                            )
                            i += 1
                    if pi == 1 and mid_hook is not None:
                        mid_hook()
                        mid_hook = None
                d_sc = dsbs[b][:, oc : oc + 1]
                evA = evpool.tile([128, 2, 512], bf16, tag="ev")
                evB = evpool.tile([128, 2, 512], bf16, tag="ev")
                nc.scalar.activation(evA, pwA, AF.Copy, scale=d_sc)
                nc.scalar.activation(evB, pwB, AF.Copy, scale=d_sc)
                # planar inverse transform: y_ev = m0+m1+m2, y_od = m1-m2-m3
                # (contiguous ops; the strided store APs re-interleave parity)
                ve = nc.vector
                ose = ospool.tile([128, 512], bf16, tag="ose")
                oso = ospool.tile([128, 512], bf16, tag="oso")
                te = gtmp.tile([128, 512], bf16, tag="te")
                to = gtmp.tile([128, 512], bf16, tag="to")
                ve.tensor_add(te, evA[:, 0], evA[:, 1])
                ve.tensor_add(ose, te, evB[:, 0])
                ve.tensor_sub(to, evA[:, 1], evB[:, 0])
                ve.tensor_sub(oso, to, evB[:, 1])
                # output transposes on the xbar, all on the idle sync ring
                eng = nc.sync
                eng.dma_start_transpose(
                    out=ob[:, :, oc * 128 : (oc + 1) * 128], in_=ose
                )
                eng.dma_start_transpose(
                    out=obo[:, :, oc * 128 : (oc + 1) * 128], in_=oso
                )
            nc.gpsimd.dma_start(out=y_par_ap(b, blk, 0), in_=ob)
            nc.gpsimd.dma_start(out=y_par_ap(b, blk, 1), in_=obo)



        # ---- emission (engine program order = schedule) ----
        style_prep(0)
        build_weights()
        start_sample(0)
        ivchunk(0, 0)
        ivchunk(0, 1)
        ivchunk(0, 2)
        # fp16 squared taps (one big DVE op) + b1 style prep
        for cc in range(2):
            nc.vector.tensor_mul(ksq[:, cc], k16[:, cc], k16[:, cc])
        style_prep(1)

        def hook0():
            demod(0)
            demod(1)

        ivchunk(0, 3)
        ivchunk(0, 4)
        conv_blk(0, 0, mid_hook=hook0)
        ivchunk(0, 5)
        ivchunk(0, 6)
        conv_blk(0, 1)
        ivchunk(0, 7)
        start_sample(1)
        ivchunk(1, 0)
        conv_blk(0, 2)
        ivchunk(1, 1)
        ivchunk(1, 2)
        conv_blk(0, 3)
        ivchunk(1, 3)
        ivchunk(1, 4)
        conv_blk(1, 0)
        ivchunk(1, 5)
        ivchunk(1, 6)
        conv_blk(1, 1)
        ivchunk(1, 7)
        conv_blk(1, 2)
        conv_blk(1, 3)

    nc.compile()
    return nc


def _get_nc():
    if "nc" not in _CACHE:
        _CACHE["nc"] = _build_nc()
    return _CACHE["nc"]


def kernel(x, style, kernel, _trace=False):
    global LAST_EXEC_NS, LAST_MEAN_EXEC_NS
    from concourse.bass_utils import run_bass_kernel_spmd

    x = np.ascontiguousarray(x, dtype=np.float32)
    style = np.ascontiguousarray(style, dtype=np.float32)
    kern = np.ascontiguousarray(kernel, dtype=np.float32)

    nc = _get_nc()
    in_maps = [
        {
            "x": x[i * BPC : (i + 1) * BPC],
            "style": style[i * BPC : (i + 1) * BPC],
            "kernel": kern,
        }
        for i in range(NCORES)
    ]
    res = run_bass_kernel_spmd(nc, in_maps, core_ids=list(range(NCORES)), trace=_trace)
    LAST_EXEC_NS = res.exec_time_ns
    LAST_MEAN_EXEC_NS = res.mean_exec_time_ns
    return np.concatenate([res.results[i]["y"] for i in range(NCORES)], axis=0)
